# revision 23
# baseline (speedup 1.0000x reference)
import contextlib
import os
import sys

os.environ.setdefault("MYCRO_LOCAL_CACHE", "1")
for _p in ("/opt/trn_rl_repo",):
    if os.path.isdir(_p) and _p not in sys.path:
        sys.path.append(_p)

import ml_dtypes
import numpy as np

import concourse.bass as bass
from concourse import bacc
import concourse.mybir as mybir
import concourse.tile as tile
from concourse.bass_utils import run_bass_kernel_spmd

FP = mybir.dt.float32
BF = mybir.dt.bfloat16
F8 = mybir.dt.float8e4
DR = mybir.MatmulPerfMode.DoubleRow
AF = mybir.ActivationFunctionType

B, N, D, H = 2, 2048, 1024, 16
NCORES = 8
GRP = 4
HPC = H // GRP
C = D // GRP
R = N // GRP
DH = D // H
SCALE = DH ** -0.5
LN_EPS = 1e-5

NT = N // 128
KD = D // 128
NS = N // 512

ATT_DT = BF
PROJ_DT = F8
STE_DT = F8


def build():
    nc = bacc.Bacc("TRN2", target_bir_lowering=False, num_devices=NCORES)

    xT_t = nc.dram_tensor("xT", [D, N], PROJ_DT, kind="ExternalInput")
    posT_t = nc.dram_tensor("posT", [C, N], FP, kind="ExternalInput")
    wq_t = nc.dram_tensor("wq", [D, C], PROJ_DT, kind="ExternalInput")
    wk_t = nc.dram_tensor("wk", [D, C], PROJ_DT, kind="ExternalInput")
    wv_t = nc.dram_tensor("wv", [D, C], PROJ_DT, kind="ExternalInput")
    wo_t = nc.dram_tensor("wo", [C, D], BF, kind="ExternalInput")
    res_t = nc.dram_tensor("resid", [R, D], FP, kind="ExternalInput")
    g_t = nc.dram_tensor("ln_g", [D], FP, kind="ExternalInput")
    bt_t = nc.dram_tensor("ln_b", [D], FP, kind="ExternalInput")
    out_t = nc.dram_tensor("out", [R, D], FP, kind="ExternalOutput")

    res_tiles = res_t.ap().rearrange("(t p) d -> t p d", p=128)
    out_tiles = out_t.ap().rearrange("(t p) d -> t p d", p=128)

    def bcast_ap(ap, parts):
        return bass.AP(tensor=ap.tensor, offset=ap.offset,
                       ap=[[0, parts]] + list(ap.ap))

    with tile.TileContext(nc) as tc, contextlib.ExitStack() as ctx:
        persist = ctx.enter_context(tc.tile_pool(name="persist", bufs=1))
        attnp = ctx.enter_context(tc.tile_pool(name="attnp", bufs=1))
        psP = ctx.enter_context(tc.tile_pool(name="psP", bufs=2, space="PSUM"))
        psO = ctx.enter_context(tc.tile_pool(name="psO", bufs=2, space="PSUM"))
        psC = ctx.enter_context(tc.tile_pool(name="psC", bufs=2, space="PSUM"))
        dram = ctx.enter_context(tc.tile_pool(name="dram", bufs=1, space="DRAM"))

        ones64 = persist.tile([1, DH], FP, tag="ones64")
        nc.vector.memset(ones64, 1.0)
        onescol = persist.tile([128, 1], FP, tag="onescol")
        nc.vector.memset(onescol, 1.0)

        sbA = ctx.enter_context(tc.tile_pool(name="sbA", bufs=3))
        sbB = ctx.enter_context(tc.tile_pool(name="sbB", bufs=2))

        ph12_ctx = contextlib.ExitStack()
        p12 = ph12_ctx.enter_context(tc.tile_pool(name="ph12", bufs=1))

        wq_sb = p12.tile([128, KD, C], PROJ_DT, tag="wq")
        wk_sb = p12.tile([128, KD, C], PROJ_DT, tag="wk")
        wv_sb = p12.tile([128, KD, C], PROJ_DT, tag="wv")
        xT_sb = p12.tile([128, KD, N], PROJ_DT, tag="xT")
        posT_sb = p12.tile([128, 2, N], FP, tag="posT")
        xT_src = xT_t.ap().rearrange("(k p) n -> p k n", p=128)
        posT_src = posT_t.ap().rearrange("(m p) n -> p m n", p=128)

        nc.sync.dma_start(out=wk_sb, in_=wk_t.ap().rearrange("(k p) c -> p k c", p=128))
        nc.sync.dma_start(out=posT_sb[:, 0, :], in_=posT_src[:, 0, :])
        for k in range(KD):
            nc.sync.dma_start(out=xT_sb[:, k, 0:512], in_=xT_src[:, k, 0:512])
        nc.sync.dma_start(out=wq_sb, in_=wq_t.ap().rearrange("(k p) c -> p k c", p=128))
        nc.sync.dma_start(out=wv_sb, in_=wv_t.ap().rearrange("(k p) c -> p k c", p=128))
        nc.sync.dma_start(out=posT_sb[:, 1, :], in_=posT_src[:, 1, :])
        for s4 in range(1, 4):
            for k in range(KD):
                nc.sync.dma_start(out=xT_sb[:, k, s4 * 512:(s4 + 1) * 512],
                                  in_=xT_src[:, k, s4 * 512:(s4 + 1) * 512])
        xT = [xT_sb[:, k, :] for k in range(KD)]
        posT = [posT_sb[:, m, :] for m in range(2)]

        wo_sb = persist.tile([128, 2, D], BF, tag="wo")
        nc.sync.dma_start(out=wo_sb, in_=wo_t.ap().rearrange("(k p) d -> p k d", p=128))
        g_sb = persist.tile([128, D], FP, tag="g")
        b_sb = persist.tile([128, D], FP, tag="b")
        nc.gpsimd.dma_start(out=g_sb, in_=bcast_ap(g_t.ap(), 128))
        nc.gpsimd.dma_start(out=b_sb, in_=bcast_ap(bt_t.ap(), 128))
        eps_sb = persist.tile([128, 1], FP, tag="eps")
        nc.vector.memset(eps_sb, LN_EPS)
        shift_sb = persist.tile([128, 1], FP, tag="shift")
        nc.vector.memset(shift_sb, -4.0)
        res_sb = persist.tile([128, NS, D], FP, tag="res")
        for s in range(NS):
            nc.sync.dma_start(out=res_sb[:, s, :], in_=res_tiles[s])

        qT = [attnp.tile([128, N], ATT_DT, name=f"qT{m}", tag=f"qT{m}") for m in range(2)]
        kpT = [attnp.tile([128, N], ATT_DT, name=f"kpT{m}", tag=f"kpT{m}") for m in range(2)]
        VP = DH + 16
        V2 = [attnp.tile([128, 2, HPC, VP], STE_DT, name=f"V{t}", tag=f"V{t}")
              for t in range(NT // 2)]

        def proj_kp(m, s):
            kp_ps = psP.tile([128, 512], FP, tag="ps", name="kp_ps")
            for k2 in range(KD // 2):
                nc.tensor.matmul(kp_ps,
                                 wk_sb[:, 2 * k2:2 * k2 + 2, m * 128:(m + 1) * 128],
                                 xT_sb[:, 2 * k2:2 * k2 + 2, s * 512:(s + 1) * 512],
                                 start=(k2 == 0), stop=(k2 == KD // 2 - 1),
                                 perf_mode=DR)
            nc.vector.tensor_add(out=kpT[m][:, s * 512:(s + 1) * 512],
                                 in0=kp_ps, in1=posT[m][:, s * 512:(s + 1) * 512])

        def proj_q(m, s):
            q_ps = psP.tile([128, 512], FP, tag="ps", name="q_ps")
            for k2 in range(KD // 2):
                nc.tensor.matmul(q_ps,
                                 wq_sb[:, 2 * k2:2 * k2 + 2, m * 128:(m + 1) * 128],
                                 xT_sb[:, 2 * k2:2 * k2 + 2, s * 512:(s + 1) * 512],
                                 start=(k2 == 0), stop=(k2 == KD // 2 - 1),
                                 perf_mode=DR)
            nc.vector.tensor_copy(out=qT[m][:, s * 512:(s + 1) * 512], in_=q_ps)

        def proj_v(t):
            v_ps = psP.tile([128, C], FP, tag="ps", name="v_ps")
            for k2 in range(KD // 2):
                nc.tensor.matmul(v_ps,
                                 xT_sb[:, 2 * k2:2 * k2 + 2, t * 128:(t + 1) * 128],
                                 wv_sb[:, 2 * k2:2 * k2 + 2, :],
                                 start=(k2 == 0), stop=(k2 == KD // 2 - 1),
                                 perf_mode=DR)
            vt = V2[t // 2][:, t % 2]
            if t % 2 == 0:
                nc.vector.memset(V2[t // 2][:, :, :, DH + 1:VP], 0.0)
            nc.vector.tensor_copy(out=vt[:, :, 0:DH],
                                  in_=v_ps.rearrange("p (h d) -> p h d", h=HPC))
            nc.vector.tensor_copy(out=vt[:, :, DH:DH + 1],
                                  in_=onescol.broadcast_to([128, HPC, 1]))

        OT = [attnp.tile([128, N], BF, name=f"OT{m}", tag=f"OT{m}") for m in range(2)]
        OTU = [attnp.tile([128, N], FP, name=f"OTU{m}", tag=f"OTU{m}") for m in range(2)]
        oph = [[dram.tile([256, D], BF, name=f"oph{s}_{h}", tag=f"oph{s}_{h}")
                for h in range(2)] for s in range(NS)]
        rsh = [[dram.tile([64, D], BF, name=f"rsh{s}_{h}", tag=f"rsh{s}_{h}")
                for h in range(2)] for s in range(NS)]

        def attention(i0, iw, hp, fillers):
            ot_e = psO.tile([128, 512], FP, tag="ot", name="ot_e")
            ot_o = psO.tile([128, 512], FP, tag="ot", name="ot_o")
            stes = {}
            stepair = None
            for jt in range(NT + 1):
                if jt < NT:
                    st = psC.tile([128, 1024], FP, tag="st", name="st")
                    nc.tensor.matmul(st[:, 0:iw],
                                     kpT[hp][0:64, jt * 128:(jt + 1) * 128],
                                     qT[hp][0:64, i0:i0 + iw],
                                     start=True, stop=True)
                    nc.tensor.matmul(st[:, iw:2 * iw],
                                     kpT[hp][64:128, jt * 128:(jt + 1) * 128],
                                     qT[hp][64:128, i0:i0 + iw],
                                     start=True, stop=True)
                    if jt % 2 == 0:
                        stepair = sbA.tile([128, 2, 1024], STE_DT, tag="ste",
                                           name="ste")
                        stes[jt // 2] = stepair
                    nc.scalar.activation(out=stepair[:, jt % 2, 0:2 * iw], in_=st[:, 0:2 * iw],
                                         func=AF.Exp, scale=SCALE, bias=shift_sb)
                for f in fillers.get(jt, ()):
                    f()
                if iw == 512 and jt >= 2 and jt % 2 == 0:
                    jj = (jt - 2) // 2
                    sp = stes.pop(jj)
                    nc.tensor.matmul(ot_e[0:DH + 1, 0:iw], V2[jj][:, :, 2 * hp, 0:DH + 1],
                                     sp[:, :, 0:iw],
                                     start=(jj == 0), stop=(jj == NT // 2 - 1),
                                     perf_mode=DR)
                    nc.tensor.matmul(ot_o[0:DH + 1, 0:iw], V2[jj][:, :, 2 * hp + 1, 0:DH + 1],
                                     sp[:, :, iw:2 * iw],
                                     start=(jj == 0), stop=(jj == NT // 2 - 1),
                                     perf_mode=DR)
                elif iw != 512 and jt >= 1:
                    j2 = jt - 1
                    sp = stes[j2 // 2]
                    nc.tensor.matmul(ot_e[0:DH + 1, 0:iw],
                                     V2[j2 // 2][:, j2 % 2, 2 * hp, 0:DH + 1],
                                     sp[:, j2 % 2, 0:iw],
                                     start=(j2 == 0), stop=(j2 == NT - 1))
                    nc.tensor.matmul(ot_o[0:DH + 1, 0:iw],
                                     V2[j2 // 2][:, j2 % 2, 2 * hp + 1, 0:DH + 1],
                                     sp[:, j2 % 2, iw:2 * iw],
                                     start=(j2 == 0), stop=(j2 == NT - 1))
                    if j2 % 2 == 1:
                        stes.pop(j2 // 2)
            jobs = []
            for par, ot in ((0, ot_e), (1, ot_o)):
                csrow = sbA.tile([1, 512], FP, tag="csrow", name="csrow", bufs=8)
                nc.vector.tensor_copy(out=csrow[:, 0:iw], in_=ot[DH:DH + 1, 0:iw])
                dst = OT[hp][par * 64:par * 64 + DH, i0:i0 + iw]
                dstu = OTU[hp][par * 64:par * 64 + DH, i0:i0 + iw]
                nc.vector.tensor_copy(out=dstu, in_=ot[0:DH, 0:iw])
                jobs.append((dst, dstu, csrow, par, iw))
            return jobs

        def normalize(jobs):
            for i in range(0, len(jobs), 2):
                rec = psP.tile([128, 512], FP, tag="ps", name="rec")
                for dst, dstu, csrow, par, iw in jobs[i:i + 2]:
                    csr = sbA.tile([1, 512], FP, tag="csr", name="csr", bufs=4)
                    nc.vector.reciprocal_approx_fast(out=csr[:, 0:iw],
                                                     in_=csrow[:, 0:iw])
                    rows = rec[par * 64:par * 64 + DH, 0:iw]
                    nc.tensor.matmul(rows, ones64, csr[:, 0:iw],
                                     start=True, stop=True)
                    nc.vector.tensor_mul(out=dst, in0=dstu, in1=rows)

        def outproj_block(s, it4):
            it = s * 4 + it4
            op_sb = sbB.tile([128, D], BF, tag="op", name="op_sb")
            for nh in range(2):
                op_ps = psP.tile([128, 512], FP, tag="ps", name="op_ps")
                for kt in range(2):
                    nc.tensor.matmul(op_ps, OT[kt][:, it * 128:(it + 1) * 128],
                                     wo_sb[:, kt, nh * 512:(nh + 1) * 512],
                                     start=(kt == 0), stop=(kt == 1))
                nc.vector.tensor_copy(out=op_sb[:, nh * 512:(nh + 1) * 512],
                                      in_=op_ps)
            h, it2 = divmod(it4, 2)
            nc.sync.dma_start(
                out=oph[s][h][:].rearrange("(t p) d -> t p d", p=128)[it2],
                in_=op_sb)

        def rs_half(s, h):
            nc.gpsimd.collective_compute(
                "ReduceScatter", mybir.AluOpType.add,
                replica_groups=[[0, 1, 2, 3], [4, 5, 6, 7]],
                ins=[oph[s][h].opt()], outs=[rsh[s][h].opt()])

        def ln(s):
            xr = sbB.tile([128, D], FP, tag="xr", name="xr")
            rs_sb = sbB.tile([128, D], BF, tag="rsld", name="rs_sb")
            nc.sync.dma_start(out=rs_sb[0:64, :], in_=rsh[s][0][:])
            nc.sync.dma_start(out=rs_sb[64:128, :], in_=rsh[s][1][:])
            nc.vector.tensor_add(out=xr, in0=rs_sb, in1=res_sb[:, s, :])
            stats = sbB.tile([128, 2, 6], FP, tag="stats", name="stats")
            mv = sbB.tile([128, 2], FP, tag="mv", name="mv")
            nc.vector.bn_stats(out=stats[:, 0, :], in_=xr[:, 0:512])
            nc.vector.bn_stats(out=stats[:, 1, :], in_=xr[:, 512:1024])
            nc.vector.bn_aggr(out=mv, in_=stats)
            nc.scalar.activation(out=mv[:, 1:2], in_=mv[:, 1:2], func=AF.Ln,
                                 bias=eps_sb, scale=1.0)
            nc.scalar.activation(out=mv[:, 1:2], in_=mv[:, 1:2], func=AF.Exp,
                                 scale=-0.5)
            nc.vector.tensor_scalar(out=xr, in0=xr,
                                    scalar1=mv[:, 0:1], scalar2=mv[:, 1:2],
                                    op0=mybir.AluOpType.subtract,
                                    op1=mybir.AluOpType.mult)
            nc.vector.tensor_mul(out=xr, in0=xr, in1=g_sb)
            nc.vector.tensor_add(out=xr, in0=xr, in1=b_sb)
            nc.sync.dma_start(out=out_tiles[s], in_=xr)

        for s4 in range(NS):
            proj_kp(0, s4)
        proj_q(0, 0)
        proj_v(0)
        proj_v(1)

        jobs = {s: [] for s in range(NS)}

        f = {jt: [(lambda t=jt + 2: proj_v(t))] for jt in range(NT - 2)}
        for i, jt in enumerate((1, 3, 5)):
            f[jt].append(lambda ss=i + 1: proj_q(0, ss))
        jobs[0] += attention(0, 512, 0, f)
        qkp1 = [(lambda ss=s4: proj_kp(1, ss)) for s4 in range(NS)] + \
               [(lambda ss=s4: proj_q(1, ss)) for s4 in range(NS)]
        for s in range(1, NS):
            chains = qkp1[(s - 1) * 3:(s - 1) * 3 + 3] if s < 3 else qkp1[6:]
            f = {2 + 5 * i: (c,) for i, c in enumerate(chains)}
            jobs[s] += attention(s * 512, 512, 0, f)

        f = {1: (lambda: normalize(jobs[3][:2]),)}
        jobs[0] += attention(0, 512, 1, f)
        for s in range(1, NS - 1):
            ss = s - 1
            f = {
                1: (lambda ss=ss: normalize(jobs[ss]),),
                4: (lambda ss=ss: outproj_block(ss, 0),),
                7: (lambda ss=ss: outproj_block(ss, 1),
                    lambda ss=ss: rs_half(ss, 0)),
                10: (lambda ss=ss: outproj_block(ss, 2),),
                13: (lambda ss=ss: outproj_block(ss, 3),
                     lambda ss=ss: rs_half(ss, 1)),
            }
            if s >= 2:
                f[15] = (lambda ss=s - 2: ln(ss),)
            jobs[s] += attention(s * 512, 512, 1, f)
        f = {
            1: (lambda: normalize(jobs[2]),),
            3: (lambda: outproj_block(2, 0),),
            5: (lambda: outproj_block(2, 1), lambda: rs_half(2, 0)),
            8: (lambda: outproj_block(2, 2),),
            11: (lambda: outproj_block(2, 3), lambda: rs_half(2, 1)),
            14: (lambda: ln(1),),
        }
        jobs[3] += attention(3 * 512, 512, 1, f)
        normalize(jobs[3][2:])
        outproj_block(3, 0)
        outproj_block(3, 1)
        rs_half(3, 0)
        outproj_block(3, 2)
        outproj_block(3, 3)
        rs_half(3, 1)
        ln(2)
        ln(3)
        ph12_ctx.close()

    nc.compile()
    return nc


_NC = None
_last_in_maps = None


def kernel(**inputs) -> np.ndarray:
    global _NC, _last_in_maps
    if _NC is None:
        _NC = build()
    nc = _NC

    q_s = np.asarray(inputs["q_s"], np.float32)
    pos = np.asarray(inputs["pos_emb"], np.float32)
    Wq = np.asarray(inputs["Wq"], np.float32)
    Wk = np.asarray(inputs["Wk"], np.float32)
    Wv = np.asarray(inputs["Wv"], np.float32)
    Wo = np.asarray(inputs["Wo"], np.float32)
    bo = np.asarray(inputs["bo"], np.float32)
    ln_g = np.asarray(inputs["ln_g"], np.float32)
    ln_b = np.asarray(inputs["ln_b"], np.float32)

    in_maps = []
    for c in range(NCORES):
        b, g = divmod(c, GRP)
        cs = slice(g * C, (g + 1) * C)
        resid = np.concatenate(
            [q_s[b][512 * s + 256 * h + 64 * g: 512 * s + 256 * h + 64 * (g + 1)]
             for s in range(NS) for h in range(2)],
            axis=0) + bo[None, :]
        bf = ml_dtypes.bfloat16
        f8 = ml_dtypes.float8_e4m3
        in_maps.append({
            "xT": np.ascontiguousarray(q_s[b].T.astype(f8)),
            "posT": np.ascontiguousarray(pos[b][:, cs].T),
            "wq": np.ascontiguousarray(Wq[:, cs].astype(f8)),
            "wk": np.ascontiguousarray(Wk[:, cs].astype(f8)),
            "wv": np.ascontiguousarray(Wv[:, cs].astype(f8)),
            "wo": np.ascontiguousarray(Wo[cs, :].astype(bf)),
            "resid": np.ascontiguousarray(resid),
            "ln_g": ln_g,
            "ln_b": ln_b,
        })

    _last_in_maps = in_maps
    res = run_bass_kernel_spmd(nc, in_maps, list(range(NCORES)))
    out = np.empty((B, N, D), np.float32)
    for c in range(NCORES):
        b, g = divmod(c, GRP)
        o = res.results[c]["out"]
        for s in range(NS):
            for h in range(2):
                out[b, 512 * s + 256 * h + 64 * g: 512 * s + 256 * h + 64 * (g + 1), :] = \
                    o[128 * s + 64 * h:128 * s + 64 * h + 64]
    return out


# revision 24
# speedup vs baseline: 1.1753x; 1.1753x over previous
import contextlib
import os
import sys

os.environ.setdefault("MYCRO_LOCAL_CACHE", "1")
for _p in ("/opt/trn_rl_repo",):
    if os.path.isdir(_p) and _p not in sys.path:
        sys.path.append(_p)

import ml_dtypes
import numpy as np

import concourse.bass as bass
from concourse import bacc
import concourse.mybir as mybir
import concourse.tile as tile
from concourse.bass_utils import run_bass_kernel_spmd

FP = mybir.dt.float32
BF = mybir.dt.bfloat16
F8 = mybir.dt.float8e4
DR = mybir.MatmulPerfMode.DoubleRow
AF = mybir.ActivationFunctionType

B, N, D, H = 2, 2048, 1024, 16
NCORES = 8
GRP = 4
HPC = H // GRP
C = D // GRP
R = N // GRP
DH = D // H
SCALE = DH ** -0.5
LN_EPS = 1e-5

NT = N // 128
KD = D // 128
NS = N // 512

ATT_DT = BF
PROJ_DT = F8
STE_DT = F8


def build():
    nc = bacc.Bacc("TRN2", target_bir_lowering=False, num_devices=NCORES)

    xT_t = nc.dram_tensor("xT", [D, N], PROJ_DT, kind="ExternalInput")
    posT_t = nc.dram_tensor("posT", [C, N], FP, kind="ExternalInput")
    wq_t = nc.dram_tensor("wq", [D, C], PROJ_DT, kind="ExternalInput")
    wk_t = nc.dram_tensor("wk", [D, C], PROJ_DT, kind="ExternalInput")
    wv_t = nc.dram_tensor("wv", [D, C], PROJ_DT, kind="ExternalInput")
    wo_t = nc.dram_tensor("wo", [C, D], BF, kind="ExternalInput")
    res_t = nc.dram_tensor("resid", [R, D], FP, kind="ExternalInput")
    g_t = nc.dram_tensor("ln_g", [D], FP, kind="ExternalInput")
    bt_t = nc.dram_tensor("ln_b", [D], FP, kind="ExternalInput")
    out_t = nc.dram_tensor("out", [R, D], FP, kind="ExternalOutput")

    res_tiles = res_t.ap().rearrange("(t p) d -> t p d", p=128)
    out_tiles = out_t.ap().rearrange("(t p) d -> t p d", p=128)

    def bcast_ap(ap, parts):
        return bass.AP(tensor=ap.tensor, offset=ap.offset,
                       ap=[[0, parts]] + list(ap.ap))

    with tile.TileContext(nc) as tc, contextlib.ExitStack() as ctx:
        persist = ctx.enter_context(tc.tile_pool(name="persist", bufs=1))
        attnp = ctx.enter_context(tc.tile_pool(name="attnp", bufs=1))
        psP = ctx.enter_context(tc.tile_pool(name="psP", bufs=2, space="PSUM"))
        psO = ctx.enter_context(tc.tile_pool(name="psO", bufs=2, space="PSUM"))
        psC = ctx.enter_context(tc.tile_pool(name="psC", bufs=2, space="PSUM"))
        dram = ctx.enter_context(tc.tile_pool(name="dram", bufs=1, space="DRAM"))

        ones64 = persist.tile([1, DH], FP, tag="ones64")
        nc.vector.memset(ones64, 1.0)
        onescol = persist.tile([128, 1], FP, tag="onescol")
        nc.vector.memset(onescol, 1.0)

        sbA = ctx.enter_context(tc.tile_pool(name="sbA", bufs=3))
        sbB = ctx.enter_context(tc.tile_pool(name="sbB", bufs=2))

        ph12_ctx = contextlib.ExitStack()
        p12 = ph12_ctx.enter_context(tc.tile_pool(name="ph12", bufs=1))

        wq_sb = p12.tile([128, KD, C], PROJ_DT, tag="wq")
        wk_sb = p12.tile([128, KD, C], PROJ_DT, tag="wk")
        wv_sb = p12.tile([128, KD, C], PROJ_DT, tag="wv")
        xT_sb = p12.tile([128, KD, N], PROJ_DT, tag="xT")
        posT_sb = p12.tile([128, 2, N], FP, tag="posT")
        xT_src = xT_t.ap().rearrange("(k p) n -> p k n", p=128)
        posT_src = posT_t.ap().rearrange("(m p) n -> p m n", p=128)

        nc.sync.dma_start(out=wk_sb, in_=wk_t.ap().rearrange("(k p) c -> p k c", p=128))
        nc.sync.dma_start(out=posT_sb[:, 0, :], in_=posT_src[:, 0, :])
        for k in range(KD):
            nc.sync.dma_start(out=xT_sb[:, k, 0:512], in_=xT_src[:, k, 0:512])
        nc.sync.dma_start(out=wq_sb, in_=wq_t.ap().rearrange("(k p) c -> p k c", p=128))
        nc.sync.dma_start(out=wv_sb, in_=wv_t.ap().rearrange("(k p) c -> p k c", p=128))
        nc.sync.dma_start(out=posT_sb[:, 1, :], in_=posT_src[:, 1, :])
        for s4 in range(1, 4):
            for k in range(KD):
                nc.sync.dma_start(out=xT_sb[:, k, s4 * 512:(s4 + 1) * 512],
                                  in_=xT_src[:, k, s4 * 512:(s4 + 1) * 512])
        xT = [xT_sb[:, k, :] for k in range(KD)]
        posT = [posT_sb[:, m, :] for m in range(2)]

        wo_sb = persist.tile([128, 2, D], BF, tag="wo")
        nc.sync.dma_start(out=wo_sb, in_=wo_t.ap().rearrange("(k p) d -> p k d", p=128))
        g_sb = persist.tile([128, D], FP, tag="g")
        b_sb = persist.tile([128, D], FP, tag="b")
        nc.gpsimd.dma_start(out=g_sb, in_=bcast_ap(g_t.ap(), 128))
        nc.gpsimd.dma_start(out=b_sb, in_=bcast_ap(bt_t.ap(), 128))
        eps_sb = persist.tile([128, 1], FP, tag="eps")
        nc.vector.memset(eps_sb, LN_EPS)
        shift_sb = persist.tile([128, 1], FP, tag="shift")
        nc.vector.memset(shift_sb, -4.0)
        res_sb = persist.tile([128, NS, D], FP, tag="res")
        for s in range(NS):
            nc.sync.dma_start(out=res_sb[:, s, :], in_=res_tiles[s])

        qT = [attnp.tile([128, N], ATT_DT, name=f"qT{m}", tag=f"qT{m}") for m in range(2)]
        kpT = [attnp.tile([128, N], ATT_DT, name=f"kpT{m}", tag=f"kpT{m}") for m in range(2)]
        VP = DH + 16
        V2 = [attnp.tile([128, 2, HPC, VP], STE_DT, name=f"V{t}", tag=f"V{t}")
              for t in range(NT // 2)]

        def proj_kp(m, s):
            kp_ps = psP.tile([128, 512], FP, tag="ps", name="kp_ps")
            for k2 in range(KD // 2):
                nc.tensor.matmul(kp_ps,
                                 wk_sb[:, 2 * k2:2 * k2 + 2, m * 128:(m + 1) * 128],
                                 xT_sb[:, 2 * k2:2 * k2 + 2, s * 512:(s + 1) * 512],
                                 start=(k2 == 0), stop=(k2 == KD // 2 - 1),
                                 perf_mode=DR)
            nc.vector.tensor_add(out=kpT[m][:, s * 512:(s + 1) * 512],
                                 in0=kp_ps, in1=posT[m][:, s * 512:(s + 1) * 512])

        def proj_q(m, s):
            q_ps = psP.tile([128, 512], FP, tag="ps", name="q_ps")
            for k2 in range(KD // 2):
                nc.tensor.matmul(q_ps,
                                 wq_sb[:, 2 * k2:2 * k2 + 2, m * 128:(m + 1) * 128],
                                 xT_sb[:, 2 * k2:2 * k2 + 2, s * 512:(s + 1) * 512],
                                 start=(k2 == 0), stop=(k2 == KD // 2 - 1),
                                 perf_mode=DR)
            nc.vector.tensor_copy(out=qT[m][:, s * 512:(s + 1) * 512], in_=q_ps)

        def proj_v(t):
            v_ps = psP.tile([128, C], FP, tag="ps", name="v_ps")
            for k2 in range(KD // 2):
                nc.tensor.matmul(v_ps,
                                 xT_sb[:, 2 * k2:2 * k2 + 2, t * 128:(t + 1) * 128],
                                 wv_sb[:, 2 * k2:2 * k2 + 2, :],
                                 start=(k2 == 0), stop=(k2 == KD // 2 - 1),
                                 perf_mode=DR)
            vt = V2[t // 2][:, t % 2]
            if t % 2 == 0:
                nc.vector.memset(V2[t // 2][:, :, :, DH + 1:VP], 0.0)
            nc.vector.tensor_copy(out=vt[:, :, 0:DH],
                                  in_=v_ps.rearrange("p (h d) -> p h d", h=HPC))
            nc.vector.tensor_copy(out=vt[:, :, DH:DH + 1],
                                  in_=onescol.broadcast_to([128, HPC, 1]))

        OT = [attnp.tile([128, N], BF, name=f"OT{m}", tag=f"OT{m}") for m in range(2)]
        OTU = [attnp.tile([128, N], FP, name=f"OTU{m}", tag=f"OTU{m}") for m in range(2)]
        oph = [[dram.tile([256, D], BF, name=f"oph{s}_{h}", tag=f"oph{s}_{h}")
                for h in range(2)] for s in range(NS)]
        rsh = [[dram.tile([64, D], BF, name=f"rsh{s}_{h}", tag=f"rsh{s}_{h}")
                for h in range(2)] for s in range(NS)]

        def attention(i0, iw, hp, fillers):
            ot_e = psO.tile([128, 512], FP, tag="ot", name="ot_e")
            ot_o = psO.tile([128, 512], FP, tag="ot", name="ot_o")
            stes = {}
            stepair = None
            for jt in range(NT + 3):
                if jt < NT:
                    st = psC.tile([128, 1024], FP, tag="st", name="st")
                    nc.tensor.matmul(st[:, 0:iw],
                                     kpT[hp][0:64, jt * 128:(jt + 1) * 128],
                                     qT[hp][0:64, i0:i0 + iw],
                                     start=True, stop=True)
                    nc.tensor.matmul(st[:, iw:2 * iw],
                                     kpT[hp][64:128, jt * 128:(jt + 1) * 128],
                                     qT[hp][64:128, i0:i0 + iw],
                                     start=True, stop=True)
                    if jt % 2 == 0:
                        stepair = sbA.tile([128, 2, 1024], STE_DT, tag="ste",
                                           name="ste")
                        stes[jt // 2] = stepair
                    nc.scalar.activation(out=stepair[:, jt % 2, 0:2 * iw], in_=st[:, 0:2 * iw],
                                         func=AF.Exp, scale=SCALE, bias=shift_sb)
                for f in fillers.get(jt, ()):
                    f()
                if iw == 512 and jt >= 4 and jt % 2 == 0:
                    jj = (jt - 4) // 2
                    sp = stes.pop(jj)
                    nc.tensor.matmul(ot_e[0:DH + 1, 0:iw], V2[jj][:, :, 2 * hp, 0:DH + 1],
                                     sp[:, :, 0:iw],
                                     start=(jj == 0), stop=(jj == NT // 2 - 1),
                                     perf_mode=DR)
                    nc.tensor.matmul(ot_o[0:DH + 1, 0:iw], V2[jj][:, :, 2 * hp + 1, 0:DH + 1],
                                     sp[:, :, iw:2 * iw],
                                     start=(jj == 0), stop=(jj == NT // 2 - 1),
                                     perf_mode=DR)
                elif iw != 512 and jt >= 1:
                    j2 = jt - 1
                    sp = stes[j2 // 2]
                    nc.tensor.matmul(ot_e[0:DH + 1, 0:iw],
                                     V2[j2 // 2][:, j2 % 2, 2 * hp, 0:DH + 1],
                                     sp[:, j2 % 2, 0:iw],
                                     start=(j2 == 0), stop=(j2 == NT - 1))
                    nc.tensor.matmul(ot_o[0:DH + 1, 0:iw],
                                     V2[j2 // 2][:, j2 % 2, 2 * hp + 1, 0:DH + 1],
                                     sp[:, j2 % 2, iw:2 * iw],
                                     start=(j2 == 0), stop=(j2 == NT - 1))
                    if j2 % 2 == 1:
                        stes.pop(j2 // 2)
            jobs = []
            for par, ot in ((0, ot_e), (1, ot_o)):
                csrow = sbA.tile([1, 512], FP, tag="csrow", name="csrow", bufs=8)
                nc.vector.tensor_copy(out=csrow[:, 0:iw], in_=ot[DH:DH + 1, 0:iw])
                dst = OT[hp][par * 64:par * 64 + DH, i0:i0 + iw]
                dstu = OTU[hp][par * 64:par * 64 + DH, i0:i0 + iw]
                nc.vector.tensor_copy(out=dstu, in_=ot[0:DH, 0:iw])
                jobs.append((dst, dstu, csrow, par, iw))
            return jobs

        def normalize(jobs):
            for i in range(0, len(jobs), 2):
                rec = psP.tile([128, 512], FP, tag="ps", name="rec")
                for dst, dstu, csrow, par, iw in jobs[i:i + 2]:
                    csr = sbA.tile([1, 512], FP, tag="csr", name="csr", bufs=4)
                    nc.vector.reciprocal_approx_fast(out=csr[:, 0:iw],
                                                     in_=csrow[:, 0:iw])
                    rows = rec[par * 64:par * 64 + DH, 0:iw]
                    nc.tensor.matmul(rows, ones64, csr[:, 0:iw],
                                     start=True, stop=True)
                    nc.vector.tensor_mul(out=dst, in0=dstu, in1=rows)

        def outproj_block(s, it4):
            it = s * 4 + it4
            op_sb = sbB.tile([128, D], BF, tag="op", name="op_sb")
            for nh in range(2):
                op_ps = psP.tile([128, 512], FP, tag="ps", name="op_ps")
                for kt in range(2):
                    nc.tensor.matmul(op_ps, OT[kt][:, it * 128:(it + 1) * 128],
                                     wo_sb[:, kt, nh * 512:(nh + 1) * 512],
                                     start=(kt == 0), stop=(kt == 1))
                nc.vector.tensor_copy(out=op_sb[:, nh * 512:(nh + 1) * 512],
                                      in_=op_ps)
            h, it2 = divmod(it4, 2)
            nc.sync.dma_start(
                out=oph[s][h][:].rearrange("(t p) d -> t p d", p=128)[it2],
                in_=op_sb)

        def rs_half(s, h):
            nc.gpsimd.collective_compute(
                "ReduceScatter", mybir.AluOpType.add,
                replica_groups=[[0, 1, 2, 3], [4, 5, 6, 7]],
                ins=[oph[s][h].opt()], outs=[rsh[s][h].opt()])

        def ln(s):
            xr = sbB.tile([128, D], FP, tag="xr", name="xr")
            rs_sb = sbB.tile([128, D], BF, tag="rsld", name="rs_sb")
            nc.sync.dma_start(out=rs_sb[0:64, :], in_=rsh[s][0][:])
            nc.sync.dma_start(out=rs_sb[64:128, :], in_=rsh[s][1][:])
            nc.vector.tensor_add(out=xr, in0=rs_sb, in1=res_sb[:, s, :])
            stats = sbB.tile([128, 2, 6], FP, tag="stats", name="stats")
            mv = sbB.tile([128, 2], FP, tag="mv", name="mv")
            nc.vector.bn_stats(out=stats[:, 0, :], in_=xr[:, 0:512])
            nc.vector.bn_stats(out=stats[:, 1, :], in_=xr[:, 512:1024])
            nc.vector.bn_aggr(out=mv, in_=stats)
            nc.scalar.activation(out=mv[:, 1:2], in_=mv[:, 1:2], func=AF.Ln,
                                 bias=eps_sb, scale=1.0)
            nc.scalar.activation(out=mv[:, 1:2], in_=mv[:, 1:2], func=AF.Exp,
                                 scale=-0.5)
            nc.vector.tensor_scalar(out=xr, in0=xr,
                                    scalar1=mv[:, 0:1], scalar2=mv[:, 1:2],
                                    op0=mybir.AluOpType.subtract,
                                    op1=mybir.AluOpType.mult)
            nc.vector.tensor_mul(out=xr, in0=xr, in1=g_sb)
            nc.vector.tensor_add(out=xr, in0=xr, in1=b_sb)
            nc.sync.dma_start(out=out_tiles[s], in_=xr)

        for s4 in range(NS):
            proj_kp(0, s4)
        proj_q(0, 0)
        proj_v(0)
        proj_v(1)

        jobs = {}
        f = {jt: [(lambda t=jt + 2: proj_v(t))] for jt in range(NT - 2)}
        for i, jt in enumerate((1, 3, 5)):
            f[jt].append(lambda ss=i + 1: proj_q(0, ss))
        jobs[0] = attention(0, 512, 0, f)
        for s4 in range(NS):
            proj_kp(1, s4)
        for s4 in range(NS):
            proj_q(1, s4)
        jobs[0] += attention(0, 512, 1, {})

        for s in range(1, NS):
            ss = s - 1
            f = {
                1: (lambda ss=ss: normalize(jobs.pop(ss)),),
                4: (lambda ss=ss: outproj_block(ss, 0),),
                7: (lambda ss=ss: outproj_block(ss, 1),
                    lambda ss=ss: rs_half(ss, 0)),
                10: (lambda ss=ss: outproj_block(ss, 2),),
                13: (lambda ss=ss: outproj_block(ss, 3),
                     lambda ss=ss: rs_half(ss, 1)),
            }
            jobs[s] = attention(s * 512, 512, 0, f)
            f1 = {}
            if s >= 2:
                f1[8] = (lambda ss=s - 2: ln(ss),)
            jobs[s] += attention(s * 512, 512, 1, f1)

        normalize(jobs.pop(NS - 1))
        outproj_block(3, 0)
        outproj_block(3, 1)
        rs_half(3, 0)
        outproj_block(3, 2)
        outproj_block(3, 3)
        rs_half(3, 1)
        ln(2)
        ln(3)
        ph12_ctx.close()

    nc.compile()
    return nc


_NC = None
_last_in_maps = None


def kernel(**inputs) -> np.ndarray:
    global _NC, _last_in_maps
    if _NC is None:
        _NC = build()
    nc = _NC

    q_s = np.asarray(inputs["q_s"], np.float32)
    pos = np.asarray(inputs["pos_emb"], np.float32)
    Wq = np.asarray(inputs["Wq"], np.float32)
    Wk = np.asarray(inputs["Wk"], np.float32)
    Wv = np.asarray(inputs["Wv"], np.float32)
    Wo = np.asarray(inputs["Wo"], np.float32)
    bo = np.asarray(inputs["bo"], np.float32)
    ln_g = np.asarray(inputs["ln_g"], np.float32)
    ln_b = np.asarray(inputs["ln_b"], np.float32)

    in_maps = []
    for c in range(NCORES):
        b, g = divmod(c, GRP)
        cs = slice(g * C, (g + 1) * C)
        resid = np.concatenate(
            [q_s[b][512 * s + 256 * h + 64 * g: 512 * s + 256 * h + 64 * (g + 1)]
             for s in range(NS) for h in range(2)],
            axis=0) + bo[None, :]
        bf = ml_dtypes.bfloat16
        f8 = ml_dtypes.float8_e4m3
        in_maps.append({
            "xT": np.ascontiguousarray(q_s[b].T.astype(f8)),
            "posT": np.ascontiguousarray(pos[b][:, cs].T),
            "wq": np.ascontiguousarray(Wq[:, cs].astype(f8)),
            "wk": np.ascontiguousarray(Wk[:, cs].astype(f8)),
            "wv": np.ascontiguousarray(Wv[:, cs].astype(f8)),
            "wo": np.ascontiguousarray(Wo[cs, :].astype(bf)),
            "resid": np.ascontiguousarray(resid),
            "ln_g": ln_g,
            "ln_b": ln_b,
        })

    _last_in_maps = in_maps
    res = run_bass_kernel_spmd(nc, in_maps, list(range(NCORES)))
    out = np.empty((B, N, D), np.float32)
    for c in range(NCORES):
        b, g = divmod(c, GRP)
        o = res.results[c]["out"]
        for s in range(NS):
            for h in range(2):
                out[b, 512 * s + 256 * h + 64 * g: 512 * s + 256 * h + 64 * (g + 1), :] = \
                    o[128 * s + 64 * h:128 * s + 64 * h + 64]
    return out


# revision 26
# speedup vs baseline: 1.1759x; 1.0006x over previous
import contextlib
import os
import sys

os.environ.setdefault("MYCRO_LOCAL_CACHE", "1")
for _p in ("/opt/trn_rl_repo",):
    if os.path.isdir(_p) and _p not in sys.path:
        sys.path.append(_p)

import ml_dtypes
import numpy as np

import concourse.bass as bass
from concourse import bacc
import concourse.mybir as mybir
import concourse.tile as tile
from concourse.bass_utils import run_bass_kernel_spmd

FP = mybir.dt.float32
BF = mybir.dt.bfloat16
F8 = mybir.dt.float8e4
DR = mybir.MatmulPerfMode.DoubleRow
AF = mybir.ActivationFunctionType

B, N, D, H = 2, 2048, 1024, 16
NCORES = 8
GRP = 4
HPC = H // GRP
C = D // GRP
R = N // GRP
DH = D // H
SCALE = DH ** -0.5
LN_EPS = 1e-5

NT = N // 128
KD = D // 128
NS = N // 512

ATT_DT = BF
PROJ_DT = F8
STE_DT = F8


def build():
    nc = bacc.Bacc("TRN2", target_bir_lowering=False, num_devices=NCORES)

    xT_t = nc.dram_tensor("xT", [D, N], PROJ_DT, kind="ExternalInput")
    posT_t = nc.dram_tensor("posT", [C, N], FP, kind="ExternalInput")
    wq_t = nc.dram_tensor("wq", [D, C], PROJ_DT, kind="ExternalInput")
    wk_t = nc.dram_tensor("wk", [D, C], PROJ_DT, kind="ExternalInput")
    wv_t = nc.dram_tensor("wv", [D, C], PROJ_DT, kind="ExternalInput")
    wo_t = nc.dram_tensor("wo", [C, D], F8, kind="ExternalInput")
    res_t = nc.dram_tensor("resid", [R, D], FP, kind="ExternalInput")
    g_t = nc.dram_tensor("ln_g", [D], FP, kind="ExternalInput")
    bt_t = nc.dram_tensor("ln_b", [D], FP, kind="ExternalInput")
    out_t = nc.dram_tensor("out", [R, D], FP, kind="ExternalOutput")

    res_tiles = res_t.ap().rearrange("(t p) d -> t p d", p=128)
    out_tiles = out_t.ap().rearrange("(t p) d -> t p d", p=128)

    def bcast_ap(ap, parts):
        return bass.AP(tensor=ap.tensor, offset=ap.offset,
                       ap=[[0, parts]] + list(ap.ap))

    with tile.TileContext(nc) as tc, contextlib.ExitStack() as ctx:
        persist = ctx.enter_context(tc.tile_pool(name="persist", bufs=1))
        attnp = ctx.enter_context(tc.tile_pool(name="attnp", bufs=1))
        psP = ctx.enter_context(tc.tile_pool(name="psP", bufs=2, space="PSUM"))
        psO = ctx.enter_context(tc.tile_pool(name="psO", bufs=2, space="PSUM"))
        psC = ctx.enter_context(tc.tile_pool(name="psC", bufs=2, space="PSUM"))
        dram = ctx.enter_context(tc.tile_pool(name="dram", bufs=1, space="DRAM"))

        ones64 = persist.tile([1, DH], FP, tag="ones64")
        nc.vector.memset(ones64, 1.0)
        onescol = persist.tile([128, 1], FP, tag="onescol")
        nc.vector.memset(onescol, 1.0)

        sbA = ctx.enter_context(tc.tile_pool(name="sbA", bufs=3))
        sbB = ctx.enter_context(tc.tile_pool(name="sbB", bufs=2))

        ph12_ctx = contextlib.ExitStack()
        p12 = ph12_ctx.enter_context(tc.tile_pool(name="ph12", bufs=1))

        wq_sb = p12.tile([128, KD, C], PROJ_DT, tag="wq")
        wk_sb = p12.tile([128, KD, C], PROJ_DT, tag="wk")
        wv_sb = p12.tile([128, KD, C], PROJ_DT, tag="wv")
        xT_sb = p12.tile([128, KD, N], PROJ_DT, tag="xT")
        posT_sb = p12.tile([128, 2, N], FP, tag="posT")
        xT_src = xT_t.ap().rearrange("(k p) n -> p k n", p=128)
        posT_src = posT_t.ap().rearrange("(m p) n -> p m n", p=128)

        qs = [nc.sync, nc.scalar, nc.gpsimd]
        nc.scalar.dma_start(out=wk_sb, in_=wk_t.ap().rearrange("(k p) c -> p k c", p=128))
        nc.gpsimd.dma_start(out=posT_sb[:, 0, :], in_=posT_src[:, 0, :])
        for k in range(KD):
            qs[k % 3].dma_start(out=xT_sb[:, k, 0:512], in_=xT_src[:, k, 0:512])
        nc.scalar.dma_start(out=wq_sb, in_=wq_t.ap().rearrange("(k p) c -> p k c", p=128))
        nc.gpsimd.dma_start(out=wv_sb, in_=wv_t.ap().rearrange("(k p) c -> p k c", p=128))
        nc.sync.dma_start(out=posT_sb[:, 1, :], in_=posT_src[:, 1, :])
        for s4 in range(1, 4):
            for k in range(KD):
                qs[(s4 * KD + k) % 3].dma_start(
                    out=xT_sb[:, k, s4 * 512:(s4 + 1) * 512],
                    in_=xT_src[:, k, s4 * 512:(s4 + 1) * 512])
        xT = [xT_sb[:, k, :] for k in range(KD)]
        posT = [posT_sb[:, m, :] for m in range(2)]

        wo_sb = persist.tile([128, 2, D], F8, tag="wo")
        nc.sync.dma_start(out=wo_sb, in_=wo_t.ap().rearrange("(k p) d -> p k d", p=128))
        g_sb = persist.tile([128, D], FP, tag="g")
        b_sb = persist.tile([128, D], FP, tag="b")
        nc.gpsimd.dma_start(out=g_sb, in_=bcast_ap(g_t.ap(), 128))
        nc.gpsimd.dma_start(out=b_sb, in_=bcast_ap(bt_t.ap(), 128))
        eps_sb = persist.tile([128, 1], FP, tag="eps")
        nc.vector.memset(eps_sb, LN_EPS)
        shift_sb = persist.tile([128, 1], FP, tag="shift")
        nc.vector.memset(shift_sb, -4.0)
        res_sb = persist.tile([128, NS, D], FP, tag="res")
        for s in range(NS):
            nc.sync.dma_start(out=res_sb[:, s, :], in_=res_tiles[s])

        qT = [attnp.tile([128, N], ATT_DT, name=f"qT{m}", tag=f"qT{m}") for m in range(2)]
        kpT = [attnp.tile([128, N], ATT_DT, name=f"kpT{m}", tag=f"kpT{m}") for m in range(2)]
        VP = DH + 16
        V2 = [attnp.tile([128, 2, HPC, VP], STE_DT, name=f"V{t}", tag=f"V{t}")
              for t in range(NT // 2)]

        def proj_kp(m, s):
            kp_ps = psP.tile([128, 512], FP, tag="ps", name="kp_ps")
            for k2 in range(KD // 2):
                nc.tensor.matmul(kp_ps,
                                 wk_sb[:, 2 * k2:2 * k2 + 2, m * 128:(m + 1) * 128],
                                 xT_sb[:, 2 * k2:2 * k2 + 2, s * 512:(s + 1) * 512],
                                 start=(k2 == 0), stop=(k2 == KD // 2 - 1),
                                 perf_mode=DR)
            nc.vector.tensor_add(out=kpT[m][:, s * 512:(s + 1) * 512],
                                 in0=kp_ps, in1=posT[m][:, s * 512:(s + 1) * 512])

        def proj_q(m, s):
            q_ps = psP.tile([128, 512], FP, tag="ps", name="q_ps")
            for k2 in range(KD // 2):
                nc.tensor.matmul(q_ps,
                                 wq_sb[:, 2 * k2:2 * k2 + 2, m * 128:(m + 1) * 128],
                                 xT_sb[:, 2 * k2:2 * k2 + 2, s * 512:(s + 1) * 512],
                                 start=(k2 == 0), stop=(k2 == KD // 2 - 1),
                                 perf_mode=DR)
            nc.vector.tensor_copy(out=qT[m][:, s * 512:(s + 1) * 512], in_=q_ps)

        def proj_v(t):
            v_ps = psP.tile([128, C], FP, tag="ps", name="v_ps")
            for k2 in range(KD // 2):
                nc.tensor.matmul(v_ps,
                                 xT_sb[:, 2 * k2:2 * k2 + 2, t * 128:(t + 1) * 128],
                                 wv_sb[:, 2 * k2:2 * k2 + 2, :],
                                 start=(k2 == 0), stop=(k2 == KD // 2 - 1),
                                 perf_mode=DR)
            vt = V2[t // 2][:, t % 2]
            if t % 2 == 0:
                nc.vector.memset(V2[t // 2][:, :, :, DH + 1:VP], 0.0)
            nc.vector.tensor_copy(out=vt[:, :, 0:DH],
                                  in_=v_ps.rearrange("p (h d) -> p h d", h=HPC))
            nc.vector.tensor_copy(out=vt[:, :, DH:DH + 1],
                                  in_=onescol.broadcast_to([128, HPC, 1]))

        OTb = attnp.tile([128, 2, N], STE_DT, name="OTb", tag="OTb")
        OTU = [attnp.tile([128, N], FP, name=f"OTU{m}", tag=f"OTU{m}") for m in range(2)]
        oph = [[dram.tile([256, D], BF, name=f"oph{s}_{h}", tag=f"oph{s}_{h}")
                for h in range(2)] for s in range(NS)]
        rsh = [[dram.tile([64, D], BF, name=f"rsh{s}_{h}", tag=f"rsh{s}_{h}")
                for h in range(2)] for s in range(NS)]

        def attention(i0, iw, hp, fillers):
            ot_e = psO.tile([128, 512], FP, tag="ot", name="ot_e")
            ot_o = psO.tile([128, 512], FP, tag="ot", name="ot_o")
            stes = {}
            stepair = None
            for jt in range(NT + 3):
                if jt < NT:
                    st = psC.tile([128, 1024], FP, tag="st", name="st")
                    nc.tensor.matmul(st[:, 0:iw],
                                     kpT[hp][0:64, jt * 128:(jt + 1) * 128],
                                     qT[hp][0:64, i0:i0 + iw],
                                     start=True, stop=True)
                    nc.tensor.matmul(st[:, iw:2 * iw],
                                     kpT[hp][64:128, jt * 128:(jt + 1) * 128],
                                     qT[hp][64:128, i0:i0 + iw],
                                     start=True, stop=True)
                    if jt % 2 == 0:
                        stepair = sbA.tile([128, 2, 1024], STE_DT, tag="ste",
                                           name="ste")
                        stes[jt // 2] = stepair
                    nc.scalar.activation(out=stepair[:, jt % 2, 0:2 * iw], in_=st[:, 0:2 * iw],
                                         func=AF.Exp, scale=SCALE, bias=shift_sb)
                for f in fillers.get(jt, ()):
                    f()
                if iw == 512 and jt >= 4 and jt % 2 == 0:
                    jj = (jt - 4) // 2
                    sp = stes.pop(jj)
                    nc.tensor.matmul(ot_e[0:DH + 1, 0:iw], V2[jj][:, :, 2 * hp, 0:DH + 1],
                                     sp[:, :, 0:iw],
                                     start=(jj == 0), stop=(jj == NT // 2 - 1),
                                     perf_mode=DR)
                    nc.tensor.matmul(ot_o[0:DH + 1, 0:iw], V2[jj][:, :, 2 * hp + 1, 0:DH + 1],
                                     sp[:, :, iw:2 * iw],
                                     start=(jj == 0), stop=(jj == NT // 2 - 1),
                                     perf_mode=DR)
                elif iw != 512 and jt >= 1:
                    j2 = jt - 1
                    sp = stes[j2 // 2]
                    nc.tensor.matmul(ot_e[0:DH + 1, 0:iw],
                                     V2[j2 // 2][:, j2 % 2, 2 * hp, 0:DH + 1],
                                     sp[:, j2 % 2, 0:iw],
                                     start=(j2 == 0), stop=(j2 == NT - 1))
                    nc.tensor.matmul(ot_o[0:DH + 1, 0:iw],
                                     V2[j2 // 2][:, j2 % 2, 2 * hp + 1, 0:DH + 1],
                                     sp[:, j2 % 2, iw:2 * iw],
                                     start=(j2 == 0), stop=(j2 == NT - 1))
                    if j2 % 2 == 1:
                        stes.pop(j2 // 2)
            jobs = []
            for par, ot in ((0, ot_e), (1, ot_o)):
                csrow = sbA.tile([1, 512], FP, tag="csrow", name="csrow", bufs=8)
                nc.vector.tensor_copy(out=csrow[:, 0:iw], in_=ot[DH:DH + 1, 0:iw])
                dst = OTb[par * 64:par * 64 + DH, hp, i0:i0 + iw]
                dstu = OTU[hp][par * 64:par * 64 + DH, i0:i0 + iw]
                nc.vector.tensor_copy(out=dstu, in_=ot[0:DH, 0:iw])
                jobs.append((dst, dstu, csrow, par, iw))
            return jobs

        def normalize(jobs):
            for i in range(0, len(jobs), 2):
                rec = psP.tile([128, 512], FP, tag="ps", name="rec")
                for dst, dstu, csrow, par, iw in jobs[i:i + 2]:
                    csr = sbA.tile([1, 512], FP, tag="csr", name="csr", bufs=4)
                    nc.vector.reciprocal_approx_fast(out=csr[:, 0:iw],
                                                     in_=csrow[:, 0:iw])
                    rows = rec[par * 64:par * 64 + DH, 0:iw]
                    nc.tensor.matmul(rows, ones64, csr[:, 0:iw],
                                     start=True, stop=True)
                    nc.vector.tensor_mul(out=dst, in0=dstu, in1=rows)

        def outproj_block(s, it4):
            it = s * 4 + it4
            op_sb = sbB.tile([128, D], BF, tag="op", name="op_sb")
            for nh in range(2):
                op_ps = psP.tile([128, 512], FP, tag="ps", name="op_ps")
                nc.tensor.matmul(op_ps, OTb[:, :, it * 128:(it + 1) * 128],
                                 wo_sb[:, :, nh * 512:(nh + 1) * 512],
                                 start=True, stop=True, perf_mode=DR)
                nc.vector.tensor_copy(out=op_sb[:, nh * 512:(nh + 1) * 512],
                                      in_=op_ps)
            h, it2 = divmod(it4, 2)
            nc.sync.dma_start(
                out=oph[s][h][:].rearrange("(t p) d -> t p d", p=128)[it2],
                in_=op_sb)

        def rs_half(s, h):
            nc.gpsimd.collective_compute(
                "ReduceScatter", mybir.AluOpType.add,
                replica_groups=[[0, 1, 2, 3], [4, 5, 6, 7]],
                ins=[oph[s][h].opt()], outs=[rsh[s][h].opt()])

        def ln(s):
            xr = sbB.tile([128, D], FP, tag="xr", name="xr")
            rs_sb = sbB.tile([128, D], BF, tag="rsld", name="rs_sb")
            nc.sync.dma_start(out=rs_sb[0:64, :], in_=rsh[s][0][:])
            nc.sync.dma_start(out=rs_sb[64:128, :], in_=rsh[s][1][:])
            nc.vector.tensor_add(out=xr, in0=rs_sb, in1=res_sb[:, s, :])
            stats = sbB.tile([128, 2, 6], FP, tag="stats", name="stats")
            mv = sbB.tile([128, 2], FP, tag="mv", name="mv")
            nc.vector.bn_stats(out=stats[:, 0, :], in_=xr[:, 0:512])
            nc.vector.bn_stats(out=stats[:, 1, :], in_=xr[:, 512:1024])
            nc.vector.bn_aggr(out=mv, in_=stats)
            nc.scalar.activation(out=mv[:, 1:2], in_=mv[:, 1:2], func=AF.Ln,
                                 bias=eps_sb, scale=1.0)
            nc.scalar.activation(out=mv[:, 1:2], in_=mv[:, 1:2], func=AF.Exp,
                                 scale=-0.5)
            nc.vector.tensor_scalar(out=xr, in0=xr,
                                    scalar1=mv[:, 0:1], scalar2=mv[:, 1:2],
                                    op0=mybir.AluOpType.subtract,
                                    op1=mybir.AluOpType.mult)
            nc.vector.tensor_mul(out=xr, in0=xr, in1=g_sb)
            nc.vector.tensor_add(out=xr, in0=xr, in1=b_sb)
            nc.sync.dma_start(out=out_tiles[s], in_=xr)

        for s4 in range(NS):
            proj_kp(0, s4)
        proj_q(0, 0)
        proj_v(0)
        proj_v(1)

        jobs = {}
        f = {jt: [(lambda t=jt + 2: proj_v(t))] for jt in range(NT - 2)}
        for i, jt in enumerate((1, 3, 5)):
            f[jt].append(lambda ss=i + 1: proj_q(0, ss))
        jobs[0] = attention(0, 512, 0, f)
        for s4 in range(NS):
            proj_kp(1, s4)
        for s4 in range(NS):
            proj_q(1, s4)
        jobs[0] += attention(0, 512, 1, {})

        for s in range(1, NS):
            ss = s - 1
            f = {
                1: (lambda ss=ss: normalize(jobs.pop(ss)),),
                4: (lambda ss=ss: outproj_block(ss, 0),),
                7: (lambda ss=ss: outproj_block(ss, 1),
                    lambda ss=ss: rs_half(ss, 0)),
                10: (lambda ss=ss: outproj_block(ss, 2),),
                13: (lambda ss=ss: outproj_block(ss, 3),
                     lambda ss=ss: rs_half(ss, 1)),
            }
            jobs[s] = attention(s * 512, 512, 0, f)
            f1 = {}
            if s >= 2:
                f1[8] = (lambda ss=s - 2: ln(ss),)
            jobs[s] += attention(s * 512, 512, 1, f1)

        normalize(jobs.pop(NS - 1))
        outproj_block(3, 0)
        outproj_block(3, 1)
        rs_half(3, 0)
        outproj_block(3, 2)
        outproj_block(3, 3)
        rs_half(3, 1)
        ln(2)
        ln(3)
        ph12_ctx.close()

    nc.compile()
    return nc


_NC = None
_last_in_maps = None


def kernel(**inputs) -> np.ndarray:
    global _NC, _last_in_maps
    if _NC is None:
        _NC = build()
    nc = _NC

    q_s = np.asarray(inputs["q_s"], np.float32)
    pos = np.asarray(inputs["pos_emb"], np.float32)
    Wq = np.asarray(inputs["Wq"], np.float32)
    Wk = np.asarray(inputs["Wk"], np.float32)
    Wv = np.asarray(inputs["Wv"], np.float32)
    Wo = np.asarray(inputs["Wo"], np.float32)
    bo = np.asarray(inputs["bo"], np.float32)
    ln_g = np.asarray(inputs["ln_g"], np.float32)
    ln_b = np.asarray(inputs["ln_b"], np.float32)

    in_maps = []
    for c in range(NCORES):
        b, g = divmod(c, GRP)
        cs = slice(g * C, (g + 1) * C)
        resid = np.concatenate(
            [q_s[b][512 * s + 256 * h + 64 * g: 512 * s + 256 * h + 64 * (g + 1)]
             for s in range(NS) for h in range(2)],
            axis=0) + bo[None, :]
        bf = ml_dtypes.bfloat16
        f8 = ml_dtypes.float8_e4m3
        in_maps.append({
            "xT": np.ascontiguousarray(q_s[b].T.astype(f8)),
            "posT": np.ascontiguousarray(pos[b][:, cs].T),
            "wq": np.ascontiguousarray(Wq[:, cs].astype(f8)),
            "wk": np.ascontiguousarray(Wk[:, cs].astype(f8)),
            "wv": np.ascontiguousarray(Wv[:, cs].astype(f8)),
            "wo": np.ascontiguousarray(Wo[cs, :].astype(f8)),
            "resid": np.ascontiguousarray(resid),
            "ln_g": ln_g,
            "ln_b": ln_b,
        })

    _last_in_maps = in_maps
    res = run_bass_kernel_spmd(nc, in_maps, list(range(NCORES)))
    out = np.empty((B, N, D), np.float32)
    for c in range(NCORES):
        b, g = divmod(c, GRP)
        o = res.results[c]["out"]
        for s in range(NS):
            for h in range(2):
                out[b, 512 * s + 256 * h + 64 * g: 512 * s + 256 * h + 64 * (g + 1), :] = \
                    o[128 * s + 64 * h:128 * s + 64 * h + 64]
    return out
